# revision 18
# baseline (speedup 1.0000x reference)
"""Trainium2 Bass kernel for nn_IterativeClassifier (B=65536, D=512, E=64, C=10, T=40).

V2: pure data parallel over 8 cores, batch-sharded, all PE streams bf16.

Math (descaled recurrence, relu positive-homogeneity, h^_t := 0.9^-t h_t):
  U      = (W1f @ W_feat) @ x           per batch column (feature phase)
  HA_0   = W1z @ z0 + U                 (PSUM accumulator, per pair-bank)
  step t: HA += G' @ h^_{t-1} + s_t U   (G' = (0.1/0.9) W1z W2, s_t = 0.1*0.9^-t)
          L  += CL @ h^_{t-1}           (CL = 0.1 CE W2)
          h^_t = relu(HA + beta_t)      (evac, ScalarE banks 0-1 / VectorE 2-3)
  logits = 0.9^39 L + biasL  (device) + 0.9^40 z0@CE.T (host term)

PE-array packing (per chunk of 4 pairs = 8 batch tiles of 512):
  - G'L pass (1 per pair per step): G' blockdiag rects (r0-1,c0-1)+(r2-3,c2-3)
    + CL rects in the free anti-diagonal cells -> L banks (2 pairs share a bank).
  - U-Latin pass (1 per step): 16 concurrent 32x32 diag-matmuls; Urot_j buffers
    hold the rotated U chunks so instr (P,j) sits at rect ((P+j)%4, j).
  - HA lives as one [128, 4*512] PSUM tile (4 banks); evac = 2 big ops/step.
"""

import ml_dtypes
import numpy as np

import concourse.bass as bass
import concourse.bacc as bacc
import concourse.mybir as mybir
import concourse.tile as tile
from concourse.bass_utils import run_bass_kernel_spmd

F32 = mybir.dt.float32
BF16 = mybir.dt.bfloat16
BF = ml_dtypes.bfloat16
AF = mybir.ActivationFunctionType
ALU = mybir.AluOpType

NCORES = 8
B, D, E, C, T = 65536, 512, 64, 10, 40
DEC, LR = 0.9, 0.1
NT = 512                      # batch columns per tile
BSH = B // NCORES             # 8192 batch rows per core
TILES = BSH // NT             # 16
PAIRS = TILES // 2            # 8
CP = 4                        # pairs per chunk
CHUNKS = PAIRS // CP          # 2
SPLIT = 1024                  # evac split point (bank boundary: S gets 0:1024)

# L-bank region start partition for (pair_in_bank, tile_in_pair):
#   pair even: A -> 64, B -> 32 ; pair odd: A -> 96, B -> 0
LREG = {(0, 0): 64, (0, 1): 32, (1, 0): 96, (1, 1): 0}


def _host_prep(x, z0, W_feat, b_feat, W1, b1, W2, b2, class_emb):
    f4 = np.float32
    W1f = W1[:, :E].astype(f4)
    W1z = W1[:, E:2 * E].astype(f4)
    w1t = W1[:, 2 * E].astype(f4)

    def dup(a):
        return np.concatenate([a, a], axis=0).astype(f4)

    Gp = (LR / DEC) * (W1z @ W2)                       # [64,64]
    CL = LR * (class_emb @ W2)                         # [10,64]
    wg = dup(Gp.T)                                     # [128, 64]
    cl10 = dup(CL.T)                                   # [128, 10]
    w1zbd = np.zeros((128, 128), f4)
    w1zbd[0:E, 0:E] = W1z.T
    w1zbd[E:128, E:128] = W1z.T
    Wu = W1f @ W_feat                                  # [64, 512]
    # wut[p, 64k+m] = Wu.T[128k+p, m]
    wut = Wu.T.reshape(4, 128, E).transpose(1, 0, 2).reshape(128, 4 * E).astype(f4)
    # sdiag[p, 32t+m] = s_t * (p%32 == m);  s_0 = 1, s_t = 0.1*0.9^-t
    svals = np.array([1.0] + [LR * DEC ** (-t) for t in range(1, T)], f4)
    eye = (np.arange(128)[:, None] % 32 == np.arange(32)[None, :]).astype(f4)
    sdiag = (eye[:, None, :] * svals[None, :, None]).reshape(128, T * 32)

    beta = np.stack([
        DEC ** (-t) * (b1 + (t / T) * w1t + (1 - DEC ** t) * (W1z @ b2) + W1f @ b_feat)
        for t in range(T)
    ]).T.astype(f4)                                    # [64, 40]
    beta = np.concatenate([beta, beta], axis=0)        # [128, 40]
    biasl = np.zeros((128, 1), f4)
    bl = ((1 - DEC ** T) * (class_emb @ b2)).astype(f4)
    for st in (0, 32, 64, 96):
        biasl[st:st + C, 0] = bl

    # x -> per-core per-tile [128, 4*NT]: x_dev[c,i,p,k*NT+n] = x[c*BSH+i*NT+n, 128k+p]
    xr = x.astype(f4).reshape(NCORES, TILES, NT, 4, 128).transpose(0, 1, 4, 3, 2)
    x_dev = np.ascontiguousarray(xr.reshape(NCORES, TILES, 128, 4 * NT)).astype(BF)
    # z0 -> per-core per-pair [128, NT]
    zr = z0.astype(f4).reshape(NCORES, PAIRS, 2, NT, E).transpose(0, 1, 2, 4, 3)
    z0_dev = np.ascontiguousarray(zr.reshape(NCORES, PAIRS, 128, NT)).astype(BF)

    consts = np.concatenate([wg, cl10, w1zbd, wut, sdiag], axis=1).astype(BF)
    fconsts = np.concatenate([beta, biasl], axis=1).astype(f4)
    # host-side z0 logits term
    hostL = (DEC ** T) * (z0.astype(f4) @ class_emb.T.astype(f4))   # [B, 10]
    return {"consts_d": consts, "fconsts_d": fconsts}, x_dev, z0_dev, hostL


def build(t_steps=T):
    nc = bacc.Bacc("TRN2", target_bir_lowering=False, debug=False)

    x_d = nc.dram_tensor("x_d", [TILES, 128, 4 * NT], BF16, kind="ExternalInput").ap()
    z0_d = nc.dram_tensor("z0_d", [PAIRS, 128, NT], BF16, kind="ExternalInput").ap()
    NCB = E + C + 128 + 4 * E + T * 32
    consts_d = nc.dram_tensor("consts_d", [128, NCB], BF16, kind="ExternalInput").ap()
    fconsts_d = nc.dram_tensor("fconsts_d", [128, T + 1], F32, kind="ExternalInput").ap()
    out_d = nc.dram_tensor("out_d", [TILES, C, NT], F32, kind="ExternalOutput").ap()

    scale_l = float(DEC ** (t_steps - 1))

    with tile.TileContext(nc) as tc:
        with (
            tc.sbuf_pool(name="consts", bufs=1) as cpool,
            tc.sbuf_pool(name="xt", bufs=8) as xpool,
            tc.sbuf_pool(name="urs", bufs=8) as upool,
            tc.sbuf_pool(name="hh", bufs=8) as hhpool,
            tc.sbuf_pool(name="z0s", bufs=4) as zpool,
            tc.sbuf_pool(name="ll", bufs=2) as llpool,
            tc.psum_pool(name="ha", bufs=4) as hapool,
            tc.psum_pool(name="misc", bufs=4) as mpool,
        ):
            const_sb = cpool.tile([128, NCB], BF16)
            nc.sync.dma_start(const_sb, consts_d)
            fconst_sb = cpool.tile([128, T + 1], F32)
            nc.sync.dma_start(fconst_sb, fconsts_d)
            o = 0
            def _sl(n):
                nonlocal o
                v = const_sb[:, o:o + n]; o += n; return v
            wg_sb = _sl(E); cl10_sb = _sl(C); w1zbd_sb = _sl(128)
            wut_sb = _sl(4 * E); sdiag_sb = _sl(T * 32)
            beta_sb = fconst_sb[:, 0:T]; biasl_sb = fconst_sb[:, T:T + 1]

            LO, HI = slice(0, 64), slice(64, 128)
            mm = nc.tensor.matmul

            for chunk in range(CHUNKS):
                t0 = chunk * TILES // CHUNKS       # first global tile of chunk
                p0 = chunk * CP                    # first global pair of chunk

                # ---- x loads (full tiles) ----
                XT = []
                for i in range(2 * CP):
                    xt = xpool.tile([128, 4 * NT], BF16, tag="xt", name=f"xt{t0+i}")
                    nc.gpsimd.dma_start(xt, x_d[t0 + i])
                    XT.append(xt)
                Z0 = []
                for P in range(CP):
                    z0t = zpool.tile([128, NT], BF16, tag="z0s", name=f"z0t{p0+P}")
                    nc.gpsimd.dma_start(z0t, z0_d[p0 + P])
                    Z0.append(z0t)

                # ---- feature phase: Urot_j, anti-diag placement, pair-groups
                #      G={0,1} -> pairs {2G, 2G+1}; row for (p, j):
                #      j<2: 2+((p+j)%2) ; j>=2: (p+j)%2   (always anti-diagonal)
                def ip(p, j):
                    return 2 + ((p + j) % 2) if j < 2 else (p + j) % 2
                UR = []
                for j in range(4):
                    ur = upool.tile([128, 2 * NT], BF16, tag="urs", name=f"ur{chunk}_{j}")
                    for G in range(2):
                        ups = mpool.tile([128, NT], F32, tag="m",
                                         name=f"ups{chunk}_{j}_{G}")
                        for k in range(4):
                            lhs = wut_sb[:, E * k + 32 * (j % 2): E * k + 32 * (j % 2) + 32]
                            for p in range(2):
                                i = ip(p, j)
                                xt = XT[2 * (2 * G + p) + (0 if j < 2 else 1)]
                                mm(ups[32 * i:32 * i + 32, :], lhs,
                                   xt[:, NT * k:NT * (k + 1)],
                                   start=(k == 0), stop=(k == 3),
                                   tile_position=(0, 32 * i), skip_group_check=True)
                        if (j + G) % 2 == 0:
                            nc.scalar.activation(ur[:, NT * G:NT * (G + 1)], ups,
                                                 AF.Copy, bias=0.0, scale=1.0)
                        else:
                            nc.vector.tensor_copy(ur[:, NT * G:NT * (G + 1)], ups)
                    UR.append(ur)

                # ---- HA (4 per-pair banks) + L (2 banks) ----
                HA = [hapool.tile([128, NT], F32, tag="ha", name=f"ha{chunk}_{P}")
                      for P in range(CP)]
                LB = []
                for b in range(2):
                    lb = mpool.tile([128, NT], F32, tag="m", name=f"lb{chunk}_{b}")
                    LB.append(lb)

                # z0 init: full-bank opener per pair
                for P in range(CP):
                    mm(HA[P], w1zbd_sb, Z0[P],
                       start=True, stop=False, tile_position=(0, 0),
                       skip_group_check=True)

                def upair(t, P, last=False):
                    sl = sdiag_sb[:, 32 * t:32 * (t + 1)]
                    G, p = P // 2, P % 2
                    for j in range(4):
                        i = ip(p, j)
                        mm(HA[P][32 * j:32 * j + 32, :],
                           sl[32 * i:32 * i + 32, :],
                           UR[j][32 * i:32 * i + 32, NT * G:NT * (G + 1)],
                           start=False, stop=(last and j == 3),
                           tile_position=(32 * i, 32 * j),
                           skip_group_check=True)

                def evac_pair(t, P, hhdst):
                    # split by pair parity so a group's two evacs run on
                    # different engines; alternate with t to balance totals
                    bia = beta_sb[:, t:t + 1]
                    if (P + t) % 2 == 0:
                        nc.scalar.activation(hhdst, HA[P], AF.Relu, bias=bia, scale=1.0)
                    else:
                        nc.vector.tensor_scalar(hhdst, HA[P], bia, 0.0, ALU.add, ALU.max)

                def lpair(hprev, P, first=False, final=False):
                    lb = LB[P // 2]
                    rA = LREG[(P % 2, 0)]
                    rB = LREG[(P % 2, 1)]
                    mm(lb[rA:rA + C, :], cl10_sb[LO, :], hprev[LO, :],
                       start=first, stop=False,
                       tile_position=(0, rA), skip_group_check=True)
                    mm(lb[rB:rB + C, :], cl10_sb[HI, :], hprev[HI, :],
                       start=first, stop=final,
                       tile_position=(64, rB), skip_group_check=True)

                # t=0: U with s=1
                HHprev = []
                for P in range(4):
                    upair(0, P)
                for P in range(4):
                    hh = hhpool.tile([128, NT], BF16, tag="hh", name=f"hh{chunk}_0_{P}")
                    evac_pair(0, P, hh)
                    HHprev.append(hh)

                # steps 1..T-1 — slot-ordered emission: diag lane (G') and
                # anti lane (L, U) interleave; consecutive slots never share
                # PE cells so the strict-FIFO issue never head-of-line blocks.
                def gpair(P):
                    mm(HA[P][LO, :], wg_sb[LO, :], HHprev[P][LO, :],
                       start=False, stop=False, tile_position=(0, 0),
                       skip_group_check=True)
                    mm(HA[P][HI, :], wg_sb[HI, :], HHprev[P][HI, :],
                       start=False, stop=False, tile_position=(64, 64),
                       skip_group_check=True)
                for t in range(1, t_steps):
                    HH = [None] * 4
                    last = t == t_steps - 1
                    # per group g: wave [U(Pa)+U(Pb)] (8 streams, full),
                    # wave [L(Pa)+L(Pb)+G'(Pa)] (6 streams), wave [G'(Pb)];
                    # 6 waves per step, no cell conflict between adjacent waves.
                    for g in range(2):
                        Pa, Pb = 2 * g, 2 * g + 1
                        upair(t, Pa, last=last)
                        upair(t, Pb, last=last)
                        gpair(Pa)
                        lpair(HHprev[Pa], Pa, first=(t == 1))
                        lpair(HHprev[Pb], Pb, first=(t == 1))
                        gpair(Pb)
                        for P in (Pa, Pb):
                            hh = hhpool.tile([128, NT], BF16, tag="hh",
                                             name=f"hh{chunk}_{t}_{P}")
                            evac_pair(t, P, hh)
                            HH[P] = hh
                    HHprev = HH

                # final L contribution from hh_{T-1}
                for P in range(4):
                    lpair(HHprev[P], P, final=True)

                # logits evac + store
                for b in range(2):
                    ll = llpool.tile([128, NT], F32, tag="ll", name=f"ll{chunk}_{b}")
                    nc.scalar.activation(ll, LB[b], AF.Identity,
                                         bias=biasl_sb[:, 0:1], scale=scale_l)
                    for P2 in range(2):          # pair index within bank
                        for ab in range(2):      # tile within pair
                            reg = LREG[(P2, ab)]
                            gt = t0 + 4 * b + 2 * P2 + ab
                            nc.sync.dma_start(out_d[gt], ll[reg:reg + C, :])
    nc.compile()
    return nc


_BUILT = {}


def _get_nc():
    if "nc" not in _BUILT:
        _BUILT["nc"] = build()
    return _BUILT["nc"]


def kernel(x, z0, W_feat, b_feat, W1, b1, W2, b2, class_emb, T_steps, **run_kw):
    x = np.asarray(x); z0 = np.asarray(z0)
    assert int(T_steps) == T
    const, x_dev, z0_dev, hostL = _host_prep(
        np.asarray(x), np.asarray(z0), np.asarray(W_feat), np.asarray(b_feat),
        np.asarray(W1), np.asarray(b1), np.asarray(W2), np.asarray(b2),
        np.asarray(class_emb))
    nc = _get_nc()
    in_maps = []
    for c in range(NCORES):
        m = dict(const)
        m["x_d"] = x_dev[c]
        m["z0_d"] = z0_dev[c]
        in_maps.append(m)
    res = run_bass_kernel_spmd(nc, in_maps, core_ids=list(range(NCORES)), **run_kw)
    outs = [r["out_d"] for r in res.results]  # each [TILES, C, NT]
    stacked = np.stack(outs)                  # [8, 16, 10, 512]
    logits = stacked.transpose(0, 1, 3, 2).reshape(B, C) + hostL
    if run_kw:
        kernel.last_result = res
    return np.ascontiguousarray(logits.astype(np.float32))


# revision 19
# speedup vs baseline: 1.0883x; 1.0883x over previous
"""Trainium2 Bass kernel for nn_IterativeClassifier (B=65536, D=512, E=64, C=10, T=40).

V2: pure data parallel over 8 cores, batch-sharded, all PE streams bf16.

Math (descaled recurrence, relu positive-homogeneity, h^_t := 0.9^-t h_t):
  U      = (W1f @ W_feat) @ x           per batch column (feature phase)
  HA_0   = W1z @ z0 + U                 (PSUM accumulator, per pair-bank)
  step t: HA += G' @ h^_{t-1} + s_t U   (G' = (0.1/0.9) W1z W2, s_t = 0.1*0.9^-t)
          L  += CL @ h^_{t-1}           (CL = 0.1 CE W2)
          h^_t = relu(HA + beta_t)      (evac, ScalarE banks 0-1 / VectorE 2-3)
  logits = 0.9^39 L + biasL  (device) + 0.9^40 z0@CE.T (host term)

PE-array packing (per chunk of 4 pairs = 8 batch tiles of 512):
  - G'L pass (1 per pair per step): G' blockdiag rects (r0-1,c0-1)+(r2-3,c2-3)
    + CL rects in the free anti-diagonal cells -> L banks (2 pairs share a bank).
  - U-Latin pass (1 per step): 16 concurrent 32x32 diag-matmuls; Urot_j buffers
    hold the rotated U chunks so instr (P,j) sits at rect ((P+j)%4, j).
  - HA lives as one [128, 4*512] PSUM tile (4 banks); evac = 2 big ops/step.
"""

import ml_dtypes
import numpy as np

import concourse.bass as bass
import concourse.bacc as bacc
import concourse.mybir as mybir
import concourse.tile as tile
from concourse.bass_utils import run_bass_kernel_spmd

F32 = mybir.dt.float32
BF16 = mybir.dt.bfloat16
BF = ml_dtypes.bfloat16
AF = mybir.ActivationFunctionType
ALU = mybir.AluOpType

NCORES = 8
B, D, E, C, T = 65536, 512, 64, 10, 40
DEC, LR = 0.9, 0.1
NT = 512                      # batch columns per tile
BSH = B // NCORES             # 8192 batch rows per core
TILES = BSH // NT             # 16
PAIRS = TILES // 2            # 8
CP = 4                        # pairs per chunk
CHUNKS = PAIRS // CP          # 2
SPLIT = 1024                  # evac split point (bank boundary: S gets 0:1024)

# L-bank region start partition for (pair_in_bank, tile_in_pair):
#   pair even: A -> 64, B -> 32 ; pair odd: A -> 96, B -> 0
LREG = {(0, 0): 64, (0, 1): 32, (1, 0): 96, (1, 1): 0}


def _host_prep(x, z0, W_feat, b_feat, W1, b1, W2, b2, class_emb):
    f4 = np.float32
    W1f = W1[:, :E].astype(f4)
    W1z = W1[:, E:2 * E].astype(f4)
    w1t = W1[:, 2 * E].astype(f4)

    def dup(a):
        return np.concatenate([a, a], axis=0).astype(f4)

    Gp = (LR / DEC) * (W1z @ W2)                       # [64,64]
    CL = LR * (class_emb @ W2)                         # [10,64]
    wg = dup(Gp.T)                                     # [128, 64]
    cl10 = dup(CL.T)                                   # [128, 10]
    w1zbd = np.zeros((128, 128), f4)
    w1zbd[0:E, 0:E] = W1z.T
    w1zbd[E:128, E:128] = W1z.T
    Wu = W1f @ W_feat                                  # [64, 512]
    # wut[p, 64k+m] = Wu.T[128k+p, m]
    wut = Wu.T.reshape(4, 128, E).transpose(1, 0, 2).reshape(128, 4 * E).astype(f4)
    # sdiag[p, 32t+m] = s_t * (p%32 == m);  s_0 = 1, s_t = 0.1*0.9^-t
    svals = np.array([1.0] + [LR * DEC ** (-t) for t in range(1, T)], f4)
    eye = (np.arange(128)[:, None] % 32 == np.arange(32)[None, :]).astype(f4)
    sdiag = (eye[:, None, :] * svals[None, :, None]).reshape(128, T * 32)

    beta = np.stack([
        DEC ** (-t) * (b1 + (t / T) * w1t + (1 - DEC ** t) * (W1z @ b2) + W1f @ b_feat)
        for t in range(T)
    ]).T.astype(f4)                                    # [64, 40]
    beta = np.concatenate([beta, beta], axis=0)        # [128, 40]
    biasl = np.zeros((128, 1), f4)
    bl = ((1 - DEC ** T) * (class_emb @ b2)).astype(f4)
    for st in (0, 32, 64, 96):
        biasl[st:st + C, 0] = bl

    # x -> per-core per-tile [128, 4*NT]: x_dev[c,i,p,k*NT+n] = x[c*BSH+i*NT+n, 128k+p]
    xr = x.astype(f4).reshape(NCORES, TILES, NT, 4, 128).transpose(0, 1, 4, 3, 2)
    x_dev = np.ascontiguousarray(xr.reshape(NCORES, TILES, 128, 4 * NT)).astype(BF)
    # z0 -> per-core per-pair [128, NT]
    zr = z0.astype(f4).reshape(NCORES, PAIRS, 2, NT, E).transpose(0, 1, 2, 4, 3)
    z0_dev = np.ascontiguousarray(zr.reshape(NCORES, PAIRS, 128, NT)).astype(BF)

    consts = np.concatenate([wg, cl10, w1zbd, wut, sdiag], axis=1).astype(BF)
    fconsts = np.concatenate([beta, biasl], axis=1).astype(f4)
    # host-side z0 logits term
    hostL = (DEC ** T) * (z0.astype(f4) @ class_emb.T.astype(f4))   # [B, 10]
    return {"consts_d": consts, "fconsts_d": fconsts}, x_dev, z0_dev, hostL


def build(t_steps=T):
    nc = bacc.Bacc("TRN2", target_bir_lowering=False, debug=False)

    x_d = nc.dram_tensor("x_d", [TILES, 128, 4 * NT], BF16, kind="ExternalInput").ap()
    z0_d = nc.dram_tensor("z0_d", [PAIRS, 128, NT], BF16, kind="ExternalInput").ap()
    NCB = E + C + 128 + 4 * E + T * 32
    consts_d = nc.dram_tensor("consts_d", [128, NCB], BF16, kind="ExternalInput").ap()
    fconsts_d = nc.dram_tensor("fconsts_d", [128, T + 1], F32, kind="ExternalInput").ap()
    out_d = nc.dram_tensor("out_d", [TILES, C, NT], F32, kind="ExternalOutput").ap()

    scale_l = float(DEC ** (t_steps - 1))

    with tile.TileContext(nc) as tc:
        with (
            tc.sbuf_pool(name="consts", bufs=1) as cpool,
            tc.sbuf_pool(name="xt", bufs=8) as xpool,
            tc.sbuf_pool(name="urs", bufs=8) as upool,
            tc.sbuf_pool(name="hh", bufs=8) as hhpool,
            tc.sbuf_pool(name="z0s", bufs=4) as zpool,
            tc.sbuf_pool(name="ll", bufs=2) as llpool,
            tc.psum_pool(name="ha", bufs=4) as hapool,
            tc.psum_pool(name="misc", bufs=4) as mpool,
        ):
            const_sb = cpool.tile([128, NCB], BF16)
            nc.sync.dma_start(const_sb, consts_d)
            fconst_sb = cpool.tile([128, T + 1], F32)
            nc.sync.dma_start(fconst_sb, fconsts_d)
            o = 0
            def _sl(n):
                nonlocal o
                v = const_sb[:, o:o + n]; o += n; return v
            wg_sb = _sl(E); cl10_sb = _sl(C); w1zbd_sb = _sl(128)
            wut_sb = _sl(4 * E); sdiag_sb = _sl(T * 32)
            beta_sb = fconst_sb[:, 0:T]; biasl_sb = fconst_sb[:, T:T + 1]

            LO, HI = slice(0, 64), slice(64, 128)
            mm = nc.tensor.matmul

            for chunk in range(CHUNKS):
                t0 = chunk * TILES // CHUNKS       # first global tile of chunk
                p0 = chunk * CP                    # first global pair of chunk

                # ---- x loads (full tiles) ----
                XT = []
                for i in range(2 * CP):
                    xt = xpool.tile([128, 4 * NT], BF16, tag="xt", name=f"xt{t0+i}")
                    nc.gpsimd.dma_start(xt, x_d[t0 + i])
                    XT.append(xt)
                Z0 = []
                for P in range(CP):
                    z0t = zpool.tile([128, NT], BF16, tag="z0s", name=f"z0t{p0+P}")
                    nc.gpsimd.dma_start(z0t, z0_d[p0 + P])
                    Z0.append(z0t)

                # ---- feature phase: Urot_j, anti-diag placement, pair-groups
                #      G={0,1} -> pairs {2G, 2G+1}; row for (p, j):
                #      j<2: 2+((p+j)%2) ; j>=2: (p+j)%2   (always anti-diagonal)
                def ip(p, j):
                    return 2 + ((p + j) % 2) if j < 2 else (p + j) % 2
                UR = []
                for j in range(4):
                    ur = upool.tile([128, 2 * NT], BF16, tag="urs", name=f"ur{chunk}_{j}")
                    for G in range(2):
                        ups = mpool.tile([128, NT], F32, tag="m",
                                         name=f"ups{chunk}_{j}_{G}")
                        for k in range(4):
                            lhs = wut_sb[:, E * k + 32 * (j % 2): E * k + 32 * (j % 2) + 32]
                            for p in range(2):
                                i = ip(p, j)
                                xt = XT[2 * (2 * G + p) + (0 if j < 2 else 1)]
                                mm(ups[32 * i:32 * i + 32, :], lhs,
                                   xt[:, NT * k:NT * (k + 1)],
                                   start=(k == 0), stop=(k == 3),
                                   tile_position=(0, 32 * i), skip_group_check=True)
                        if (j + G) % 2 == 0:
                            nc.scalar.activation(ur[:, NT * G:NT * (G + 1)], ups,
                                                 AF.Copy, bias=0.0, scale=1.0)
                        else:
                            nc.vector.tensor_copy(ur[:, NT * G:NT * (G + 1)], ups)
                    UR.append(ur)

                # ---- HA (4 per-pair banks) + L (2 banks) ----
                HA = [hapool.tile([128, NT], F32, tag="ha", name=f"ha{chunk}_{P}")
                      for P in range(CP)]
                LB = []
                for b in range(2):
                    lb = mpool.tile([128, NT], F32, tag="m", name=f"lb{chunk}_{b}")
                    LB.append(lb)

                # z0 init: full-bank opener per pair
                for P in range(CP):
                    mm(HA[P], w1zbd_sb, Z0[P],
                       start=True, stop=False, tile_position=(0, 0),
                       skip_group_check=True)

                def upair(t, P, last=False):
                    sl = sdiag_sb[:, 32 * t:32 * (t + 1)]
                    G, p = P // 2, P % 2
                    for j in range(4):
                        i = ip(p, j)
                        mm(HA[P][32 * j:32 * j + 32, :],
                           sl[32 * i:32 * i + 32, :],
                           UR[j][32 * i:32 * i + 32, NT * G:NT * (G + 1)],
                           start=False, stop=(last and j == 3),
                           tile_position=(32 * i, 32 * j),
                           skip_group_check=True)

                def evac_pair(t, P, hhdst):
                    # split by pair parity so a group's two evacs run on
                    # different engines; alternate with t to balance totals
                    bia = beta_sb[:, t:t + 1]
                    if (P + t) % 2 == 0:
                        nc.scalar.activation(hhdst, HA[P], AF.Relu, bias=bia, scale=1.0)
                    else:
                        nc.vector.tensor_scalar(hhdst, HA[P], bia, 0.0, ALU.add, ALU.max)

                def lpair(hprev, P, first=False, final=False):
                    lb = LB[P // 2]
                    rA = LREG[(P % 2, 0)]
                    rB = LREG[(P % 2, 1)]
                    mm(lb[rA:rA + C, :], cl10_sb[LO, :], hprev[LO, :],
                       start=first, stop=False,
                       tile_position=(0, rA), skip_group_check=True)
                    mm(lb[rB:rB + C, :], cl10_sb[HI, :], hprev[HI, :],
                       start=first, stop=final,
                       tile_position=(64, rB), skip_group_check=True)

                # t=0: U with s=1
                HHprev = []
                for P in range(4):
                    upair(0, P)
                for P in range(4):
                    hh = hhpool.tile([128, NT], BF16, tag="hh", name=f"hh{chunk}_0_{P}")
                    evac_pair(0, P, hh)
                    HHprev.append(hh)

                # steps 1..T-1 — slot-ordered emission: diag lane (G') and
                # anti lane (L, U) interleave; consecutive slots never share
                # PE cells so the strict-FIFO issue never head-of-line blocks.
                def gpair(P):
                    mm(HA[P][LO, :], wg_sb[LO, :], HHprev[P][LO, :],
                       start=False, stop=False, tile_position=(0, 0),
                       skip_group_check=True)
                    mm(HA[P][HI, :], wg_sb[HI, :], HHprev[P][HI, :],
                       start=False, stop=False, tile_position=(64, 64),
                       skip_group_check=True)
                for t in range(1, t_steps):
                    HH = [None] * 4
                    last = t == t_steps - 1
                    # per group g: wave [U(Pa)+U(Pb)] (8 streams, full),
                    # wave [L(Pa)+L(Pb)+G'(Pa)] (6 streams), wave [G'(Pb)];
                    # 6 waves per step, no cell conflict between adjacent waves.
                    for g in range(2):
                        Pa, Pb = 2 * g, 2 * g + 1
                        upair(t, Pa, last=last)
                        upair(t, Pb, last=last)
                        lpair(HHprev[Pa], Pa, first=(t == 1))
                        lpair(HHprev[Pb], Pb, first=(t == 1))
                        gpair(Pa)
                        gpair(Pb)
                        for P in (Pa, Pb):
                            hh = hhpool.tile([128, NT], BF16, tag="hh",
                                             name=f"hh{chunk}_{t}_{P}")
                            evac_pair(t, P, hh)
                            HH[P] = hh
                    HHprev = HH

                # final L contribution from hh_{T-1}
                for P in range(4):
                    lpair(HHprev[P], P, final=True)

                # logits evac + store
                for b in range(2):
                    ll = llpool.tile([128, NT], F32, tag="ll", name=f"ll{chunk}_{b}")
                    nc.scalar.activation(ll, LB[b], AF.Identity,
                                         bias=biasl_sb[:, 0:1], scale=scale_l)
                    for P2 in range(2):          # pair index within bank
                        for ab in range(2):      # tile within pair
                            reg = LREG[(P2, ab)]
                            gt = t0 + 4 * b + 2 * P2 + ab
                            nc.sync.dma_start(out_d[gt], ll[reg:reg + C, :])
    nc.compile()
    return nc


_BUILT = {}


def _get_nc():
    if "nc" not in _BUILT:
        _BUILT["nc"] = build()
    return _BUILT["nc"]


def kernel(x, z0, W_feat, b_feat, W1, b1, W2, b2, class_emb, T_steps, **run_kw):
    x = np.asarray(x); z0 = np.asarray(z0)
    assert int(T_steps) == T
    const, x_dev, z0_dev, hostL = _host_prep(
        np.asarray(x), np.asarray(z0), np.asarray(W_feat), np.asarray(b_feat),
        np.asarray(W1), np.asarray(b1), np.asarray(W2), np.asarray(b2),
        np.asarray(class_emb))
    nc = _get_nc()
    in_maps = []
    for c in range(NCORES):
        m = dict(const)
        m["x_d"] = x_dev[c]
        m["z0_d"] = z0_dev[c]
        in_maps.append(m)
    res = run_bass_kernel_spmd(nc, in_maps, core_ids=list(range(NCORES)), **run_kw)
    outs = [r["out_d"] for r in res.results]  # each [TILES, C, NT]
    stacked = np.stack(outs)                  # [8, 16, 10, 512]
    logits = stacked.transpose(0, 1, 3, 2).reshape(B, C) + hostL
    if run_kw:
        kernel.last_result = res
    return np.ascontiguousarray(logits.astype(np.float32))
